# revision 5
# baseline (speedup 1.0000x reference)
"""Trainium2 Bass kernel for nn_CMCI_Mamba — v2 (engine-rebalanced).

Data-parallel over the 2B=8 mamba streams (1 per core), 2 chained layers
per launch, d-major layout (d_inner on partitions, time on free axis).

v2 vs baseline:
- fp16 tiles everywhere (host pre-casts inputs / params, upcasts outputs)
- silu fused into ACT Swish evictions (no sigmoid+mul pairs)
- depthwise conv on PE via diagonal-lhsT accumulated matmuls
- x-projection computed once (all 36 rows); per-state B/C row broadcasts
  are one-hot selection matmuls reading the 36-row tile
- per-state products on DVE; the sum over states runs on PE as
  identity-matmul PSUM accumulation
- scans paired: one tensor_tensor_scan of FD=4096 covers 2 states
  (decay column at the segment boundary zeroed to reset the recurrence)
"""
import sys
import numpy as np
from contextlib import ExitStack

for _p in ("/opt/trn_rl_repo",):
    if _p not in sys.path:
        sys.path.insert(0, _p)

import concourse.bass as bass
import concourse.bacc as bacc
import concourse.tile as tile
from concourse import mybir
from concourse import bass_utils

T, DM, DI, DS, DR, K, NL = 2048, 64, 128, 16, 4, 4, 2
B, C = 4, 2048
XR = DR + 2 * DS  # 36 rows of the x-projection
FP = mybir.dt.float32
FH = mybir.dt.float16
AX = mybir.AluOpType
AF = mybir.ActivationFunctionType

# fp16 param blob column layout, (128, _FW) per layer
_F_INW = 0                    # [0:64, 0:256]   in_wT (x cols 0:128, z cols 128:256)
_F_DIAG = 256                 # [:, 256:768]    conv diag_k (4 x 128)
_F_XPW = 768                  # [0:128, 768:804]  xp_wT (128, 36)
_F_DTW = 804                  # [0:4, 804:932]  dt_wT (4, 128)
_F_OUTW = 932                 # [:, 932:996]    out_wT (128, 64)
_F_OH = 996                   # one-hot B/C selectors for the MM-path pairs
_N_MM_PAIRS = 0               # pairs < this broadcast via PE+ACT, rest via DMA
_F_ID = _F_OH + 4 * _N_MM_PAIRS * 128
_FW = _F_ID + 128

# fp32 vector blob (128, 19): conv_b, dt_b, D, Aneg[16]
_V_CONVB, _V_DTB, _V_D, _V_ANEG = 0, 1, 2, 3
_VW = 19


def _pack_fh(raw, l):
    fh = np.zeros((DI, _FW), np.float16)
    fh[:DM, _F_INW:_F_INW + 2 * DI] = raw["in_w"][l].T
    for k in range(K):
        d0 = _F_DIAG + k * DI
        fh[:, d0:d0 + DI][np.arange(DI), np.arange(DI)] = raw["conv_w"][l][:, k]
    fh[:, _F_XPW:_F_XPW + XR] = raw["xp_w"][l].T
    fh[:DR, _F_DTW:_F_DTW + DI] = raw["dt_w"][l].T
    fh[:, _F_OUTW:_F_OUTW + DM] = raw["out_w"][l].T
    for p in range(_N_MM_PAIRS):
        for r in range(4):  # B(s0), B(s1), C(s0), C(s1)
            row = DR + 2 * p + (r & 1) + (DS if r >= 2 else 0)
            c0 = _F_OH + (4 * p + r) * DI
            fh[row, c0:c0 + DI] = 1.0
    fh[:, _F_ID:_F_ID + DI][np.arange(DI), np.arange(DI)] = 1.0
    return fh


def _pack_fv(raw, l):
    fv = np.zeros((DI, _VW), np.float32)
    fv[:, _V_CONVB] = raw["conv_b"][l]
    fv[:, _V_DTB] = raw["dt_b"][l]
    fv[:, _V_D] = raw["D"][l]
    fv[:, _V_ANEG:_V_ANEG + DS] = -np.exp(raw["A_log"][l])
    return fv


def _build_kernel(ctx, tc, u0T, fhs, fvs, xdbl_ds, outs):
    nc = tc.nc
    NCH = 4
    CF = T // NCH  # 512

    const = ctx.enter_context(tc.tile_pool(name="const", bufs=1))
    big = ctx.enter_context(tc.tile_pool(name="big", bufs=1))
    ub = ctx.enter_context(tc.tile_pool(name="ub", bufs=2))
    sl = ctx.enter_context(tc.tile_pool(name="sl", bufs=2))
    bc = ctx.enter_context(tc.tile_pool(name="bc", bufs=5))
    ps1 = ctx.enter_context(tc.tile_pool(name="ps1", bufs=4, space="PSUM"))
    psy = ctx.enter_context(tc.tile_pool(name="psy", bufs=1, space="PSUM"))

    # DMA issue order = first-use order: the input and layer-0's in_proj
    # weights head the ring queue; layer-1 params aren't needed for ~130us.
    u_t = ub.tile([DM, T], FH, tag="u", name="u_in")
    nc.sync.dma_start(u_t[:], u0T[:])
    fh = []
    fv = []
    for l in range(NL):
        t = const.tile([DI, _FW], FH, tag=f"fh{l}", name=f"fh{l}")
        if l == 0:
            nc.sync.dma_start(t[:, 0:_F_DIAG], fhs[l][:, 0:_F_DIAG])
            nc.sync.dma_start(t[:, _F_DIAG:], fhs[l][:, _F_DIAG:])
        else:
            nc.sync.dma_start(t[:], fhs[l][:])
        fh.append(t)
        v = const.tile([DI, _VW], FP, tag=f"fv{l}", name=f"fv{l}")
        nc.sync.dma_start(v[:], fvs[l][:])
        fv.append(v)

    for l in range(NL):
        h = fh[l]
        v = fv[l]
        in_wT = h[0:DM, _F_INW:_F_INW + 2 * DI]
        xp_wT = h[0:DI, _F_XPW:_F_XPW + XR]
        dt_wT = h[0:DR, _F_DTW:_F_DTW + DI]
        out_wT = h[:, _F_OUTW:_F_OUTW + DM]
        ident = h[:, _F_ID:_F_ID + DI]
        convb = v[:, _V_CONVB:_V_CONVB + 1]
        dt_b = v[:, _V_DTB:_V_DTB + 1]
        Dvec = v[:, _V_D:_V_D + 1]

        # ---- stage 1: projections + conv ----
        xpre = big.tile([DI, T + K - 1], FH, tag="xpre", name=f"xpre{l}")
        zs = big.tile([DI, T], FH, tag="zs", name=f"zs{l}")
        xact = big.tile([DI, T], FH, tag="xact", name=f"xact{l}")
        xdbl = big.tile([XR, T], FH, tag="xdbl", name=f"xdbl{l}")
        ev = big.tile([DI, T], FP, tag="ev", name=f"ev{l}")
        delta16 = big.tile([DI, T], FH, tag="delta", name=f"delta{l}")
        dx16 = big.tile([DI, T], FH, tag="dx", name=f"dx{l}")
        nc.gpsimd.memset(xpre[:, 0:K - 1], 0.0)
        for c in range(NCH):
            cs = slice(c * CF, (c + 1) * CF)
            mmx = ps1.tile([DI, CF], FP, tag="mm", name=f"mmx{l}_{c}")
            nc.tensor.matmul(mmx[:], in_wT[:, 0:DI], u_t[:, cs],
                             start=True, stop=True)
            nc.scalar.activation(xpre[:, K - 1 + c * CF:K - 1 + (c + 1) * CF],
                                 mmx[:], AF.Copy)
            mmz = ps1.tile([DI, CF], FP, tag="mm", name=f"mmz{l}_{c}")
            nc.tensor.matmul(mmz[:], in_wT[:, DI:2 * DI], u_t[:, cs],
                             start=True, stop=True)
            nc.scalar.activation(zs[:, cs], mmz[:], AF.Silu)
        for c in range(NCH):
            cs = slice(c * CF, (c + 1) * CF)
            cps = ps1.tile([DI, CF], FP, tag="mm", name=f"cps{l}_{c}")
            for k in range(K):
                nc.tensor.matmul(cps[:], h[:, _F_DIAG + k * DI:_F_DIAG + (k + 1) * DI],
                                 xpre[:, k + c * CF:k + c * CF + CF],
                                 start=(k == 0), stop=(k == K - 1))
            nc.scalar.activation(xact[:, cs], cps[:], AF.Silu, bias=convb)
        for c in range(NCH):
            cs = slice(c * CF, (c + 1) * CF)
            mmp = ps1.tile([DI, CF], FP, tag="mm", name=f"mmp{l}_{c}")
            nc.tensor.matmul(mmp[0:XR, :], xp_wT, xact[:, cs],
                             start=True, stop=True)
            nc.scalar.activation(xdbl[:, cs], mmp[0:XR, :], AF.Copy)
        # stage the B rows in DRAM, pair-interleaved [B2p B2p+1 C2p C2p+1]
        # (C rows staged inside the pair loop, after pair 0's B broadcast,
        # so the head B broadcast isn't queued behind them on the rings)
        nc.sync.dma_start(xdbl_ds[l][:, 0:2 * T], xdbl[DR:DR + DS, :])
        for c in range(NCH):
            cs = slice(c * CF, (c + 1) * CF)
            mmd = ps1.tile([DI, CF], FP, tag="mm", name=f"mmd{l}_{c}")
            nc.tensor.matmul(mmd[:], dt_wT, xdbl[0:DR, cs],
                             start=True, stop=True)
            nc.scalar.activation(ev[:, cs], mmd[:], AF.Exp, bias=dt_b)
        nc.scalar.activation(delta16[:], ev[:], AF.Ln, bias=1.0)
        nc.vector.tensor_mul(dx16[:], delta16[:], xact[:])

        # ---- SSM: 8 pairs of states ----
        y_ps = psy.tile([DI, T], FP, tag="y", name=f"y{l}")
        for p in range(DS // 2):
            s0 = 2 * p
            # bcrep layout: [B(s0) | B(s0+1) | C(s0) | C(s0+1)], T cols each
            bcrep = bc.tile([DI, 4 * T], FH, tag="bcrep", name=f"bcrep{l}_{p}")
            if p < _N_MM_PAIRS:
                for r in range(4):
                    oh = h[0:XR, _F_OH + (4 * p + r) * DI:
                           _F_OH + (4 * p + r + 1) * DI]
                    for c in range(NCH):
                        bps = ps1.tile([DI, CF], FP, tag="mm",
                                       name=f"bc{l}_{p}_{r}_{c}")
                        nc.tensor.matmul(bps[:], oh,
                                         xdbl[:, c * CF:(c + 1) * CF],
                                         start=True, stop=True)
                        nc.scalar.activation(
                            bcrep[:, r * T + c * CF:r * T + (c + 1) * CF],
                            bps[:], AF.Copy)
            elif p < 2:
                # head pairs: land the B half first so dBu can start sooner
                bsrc = xdbl_ds[l][p:p + 1, 0:2 * T]
                nc.sync.dma_start(bcrep[:, 0:2 * T],
                                  bsrc.broadcast_to((DI, 2 * T)))
                if p == 0:
                    nc.sync.dma_start(xdbl_ds[l][:, 2 * T:4 * T],
                                      xdbl[DR + DS:XR, :])
                csrc = xdbl_ds[l][p:p + 1, 2 * T:4 * T]
                nc.sync.dma_start(bcrep[:, 2 * T:4 * T],
                                  csrc.broadcast_to((DI, 2 * T)))
            else:
                src = xdbl_ds[l][p:p + 1, :]
                nc.sync.dma_start(bcrep[:], src.broadcast_to((DI, 4 * T)))
            brep = bcrep[:, 0:2 * T]
            crep = bcrep[:, 2 * T:4 * T]
            dA2 = sl.tile([DI, 2 * T], FH, tag="dA2", name=f"dA2{l}_{p}")
            for q in range(2):
                nc.scalar.activation(dA2[:, q * T:(q + 1) * T], delta16[:],
                                     AF.Exp,
                                     scale=v[:, _V_ANEG + s0 + q:_V_ANEG + s0 + q + 1])
            nc.gpsimd.memset(dA2[:, T:T + 1], 0.0)
            dBu2 = sl.tile([DI, 2 * T], FH, tag="dBu2", name=f"dBu2{l}_{p}")
            nc.vector.tensor_mul(
                dBu2[:].rearrange("p (s t) -> p s t", s=2),
                dx16[:].unsqueeze(1).broadcast_to((DI, 2, T)),
                brep.rearrange("p (s t) -> p s t", s=2))
            hs2 = sl.tile([DI, 2 * T], FH, tag="hs2", name=f"hs2{l}_{p}")
            nc.vector.tensor_tensor_scan(hs2[:], dA2[:], dBu2[:], 0.0,
                                         AX.mult, AX.add)
            hsC2 = sl.tile([DI, 2 * T], FH, tag="hsC2", name=f"hsC2{l}_{p}")
            nc.vector.tensor_mul(hsC2[:], hs2[:], crep[:])
            for q in range(2):
                for c in range(NCH):
                    nc.tensor.matmul(y_ps[:, c * CF:(c + 1) * CF], ident,
                                     hsC2[:, q * T + c * CF:q * T + (c + 1) * CF],
                                     start=(p == 0 and q == 0),
                                     stop=(p == DS // 2 - 1 and q == 1),
                                     skip_group_check=True)

        # ---- tail: y = (yacc + D*x) * silu(z); out projection ----
        y16 = big.tile([DI, T], FH, tag="y16", name=f"y16{l}")
        ydx = big.tile([DI, T], FH, tag="ydx", name=f"ydx{l}")
        yf = big.tile([DI, T], FH, tag="yf", name=f"yf{l}")
        for hf in range(2):
            hs_ = slice(hf * 1024, (hf + 1) * 1024)
            nc.scalar.activation(y16[:, hs_], y_ps[:, hs_], AF.Copy)
            nc.vector.tensor_scalar_mul(ydx[:, hs_], xact[:, hs_], Dvec)
            nc.vector.tensor_add(ydx[:, hs_], ydx[:, hs_], y16[:, hs_])
            nc.vector.tensor_mul(yf[:, hs_], ydx[:, hs_], zs[:, hs_])

        o_t = ub.tile([DM, T], FH, tag="u", name=f"o{l}")
        for c in range(NCH):
            cs = slice(c * CF, (c + 1) * CF)
            omm = ps1.tile([DM, CF], FP, tag="mm", name=f"omm{l}_{c}")
            nc.tensor.matmul(omm[:], out_wT, yf[:, cs], start=True, stop=True)
            nc.scalar.activation(o_t[:, cs], omm[:], AF.Copy)
        nc.sync.dma_start(outs[l][:], o_t[:])
        u_t = o_t


def build_program():
    nc = bacc.Bacc("TRN2", target_bir_lowering=False, debug=False)
    u0T = nc.dram_tensor("u0T", [DM, T], FH, kind="ExternalInput").ap()
    fhs = [nc.dram_tensor(f"fh_l{l}", [DI, _FW], FH,
                          kind="ExternalInput").ap() for l in range(NL)]
    fvs = [nc.dram_tensor(f"fv_l{l}", [DI, _VW], FP,
                          kind="ExternalInput").ap() for l in range(NL)]
    xdbl_ds = [nc.dram_tensor(f"xdbl_d{l}", [DS // 2, 4 * T], FH,
                              kind="Internal").ap() for l in range(NL)]
    outs = [nc.dram_tensor(f"o{l + 1}T", [DM, T], FH,
                           kind="ExternalOutput").ap() for l in range(NL)]
    with tile.TileContext(nc) as tc:
        with ExitStack() as ctx:
            _build_kernel(ctx, tc, u0T, fhs, fvs, xdbl_ds, outs)
    nc.compile()
    return nc


_PROG = None


def _get_prog():
    global _PROG
    if _PROG is None:
        _PROG = build_program()
    return _PROG


def _run_launch(u_list_T, raw, trace=False, trace_kwargs=None):
    """u_list_T: list of 8 arrays (64, 2048) fp16. raw: param dict (np).
    Returns (o1_list, o2_list) fp16 arrays and the raw result."""
    nc = _get_prog()
    fhs = [_pack_fh(raw, l) for l in range(NL)]
    fvs = [_pack_fv(raw, l) for l in range(NL)]
    in_maps = []
    for b in range(8):
        in_maps.append({
            "u0T": np.ascontiguousarray(u_list_T[b], np.float16),
            "fh_l0": fhs[0], "fh_l1": fhs[1],
            "fv_l0": fvs[0], "fv_l1": fvs[1],
        })
    res = bass_utils.run_bass_kernel_spmd(
        nc, in_maps, core_ids=list(range(8)), trace=trace,
        **(trace_kwargs or {}))
    o1 = [res.results[b]["o1T"] for b in range(8)]
    o2 = [res.results[b]["o2T"] for b in range(8)]
    return o1, o2, res


def kernel(**inputs):
    inp = {k: np.asarray(v, np.float32) for k, v in inputs.items()}
    Ms = inp["Ms_feature"]
    Pan = inp["Pan_feature"]
    h = C // 2
    names = ("in_w", "conv_w", "conv_b", "xp_w", "dt_w", "dt_b",
             "A_log", "D", "out_w")
    rawa = {n: inp["a_" + n] for n in names}
    rawb = {n: inp["b_" + n] for n in names}

    cf1 = np.concatenate([Ms[:, :h], Pan[:, h:]], axis=1)
    cf2 = np.concatenate([Pan[:, :h], Ms[:, h:]], axis=1)
    u_list = [cf1[b].T for b in range(B)] + [cf2[b].T for b in range(B)]
    o1, o2, _ = _run_launch(u_list, rawa)
    cf1_1 = np.stack([o1[b].T.astype(np.float32) for b in range(B)])
    cf2_1 = np.stack([o1[B + b].T.astype(np.float32) for b in range(B)])
    cf1_2 = np.stack([o2[b].T.astype(np.float32) for b in range(B)])
    cf2_2 = np.stack([o2[B + b].T.astype(np.float32) for b in range(B)])
    Ms1 = np.maximum((cf1_1 + cf2_1) * 0.5 + Ms, 0.0)
    Ms2 = np.maximum((cf1_2 + cf2_2) * 0.5 + Ms1, 0.0)

    cf3 = np.stack([Pan[:, ::2], Ms2[:, 1::2]], axis=2).reshape(B, C, DM)
    cf4 = np.stack([Ms2[:, ::2], Pan[:, 1::2]], axis=2).reshape(B, C, DM)
    u_list = [cf3[b].T for b in range(B)] + [cf4[b].T for b in range(B)]
    o1, o2, _ = _run_launch(u_list, rawb)
    cf3_1 = np.stack([o1[b].T.astype(np.float32) for b in range(B)])
    cf4_1 = np.stack([o1[B + b].T.astype(np.float32) for b in range(B)])
    cf3_2 = np.stack([o2[b].T.astype(np.float32) for b in range(B)])
    cf4_2 = np.stack([o2[B + b].T.astype(np.float32) for b in range(B)])
    Pan1 = np.maximum((cf3_1 + cf4_1) * 0.5 + Pan, 0.0)
    Pan2 = np.maximum((cf3_2 + cf4_2) * 0.5 + Pan1, 0.0)
    return Ms2, Pan2
